# revision 26
# baseline (speedup 1.0000x reference)
"""Trainium2 Bass kernel for ConeProjection (v3).

Math (per batch element b):
    W     = [R[:,0], R[:,1], t - eyes]          (3 rows)
    d_a   = v . W_a          (unnormalized)
    G_ac  = W_a . W_c
    inv2  = 1 / ||v||^2
    s     = (d d^T) * inv2 - alpha * G          (6 unique entries)
    out[k] = s . q[k],  q[k] = [x^2, y^2, 1, 2xy, 2x, 2y]  (169 grid pts)

Strategy: pure data-parallel over 8 NeuronCores (batch 131072 -> 16384/core).
Per core, partition p holds batch [p*NI, (p+1)*NI); within-partition index i.
Inputs load via HWDGE as fp32 (one DMA per tensor; R split for ramp); the
first multiply level reads fp32 and writes fp16. Sigma entries are written
directly into a transpose-ready padded layout (24 used / 32 cols per group of
4 i's), so PE transposes [128,128] slices straight out of the elementwise
output. Each block = 16 i's -> 1 transpose + 4 row-tiled matmul groups
(K=24, N=2x338 fp32 PSUM); per-group PSUM->SBUF copies alternate DVE/ACT
(the dominant engine cost), elementwise runs mostly on DVE/ACT with w2 on
Pool. One contiguous 692KB output DMA per block; the timing loop uses
staggered semaphore resets to soften the For_i all-engine barrier. Output
returns fp16, upcast to fp32 on the host. Measured: ~39.3us/pass loop metric,
~20us marginal (DMA roofline ~19us: 1.2MB in + 5.3MB out @ ~360GB/s/core).
"""

from contextlib import ExitStack, nullcontext

import numpy as np

import concourse.bass as bass
import concourse.bacc as bacc
import concourse.tile as tile
from concourse import mybir
from concourse.bass_utils import run_bass_kernel_spmd

N_CORES = 8
B = 131072
BC = B // N_CORES          # 16384 per core
P = 128                    # partitions
NI = BC // P               # 128 within-partition batch indices
KG = 169                   # grid points
F32 = mybir.dt.float32
F16 = mybir.dt.float16

GROUP = 4                  # i's per matmul group; K = 6*GROUP = 24
GPB = 4                    # groups per block (one [128,128] PE transpose)
IPB = GROUP * GPB          # 16 i per block
N_BLOCKS = NI // IPB       # 8
NMM = GROUP * KG           # 676 matmul free size
NMH = NMM // 2             # 338: matmul N split so fp32 out fits a PSUM bank
GW = 32                    # padded cols per group (24 used)

CFG = dict(
    CHUNKS=(1, 1, 1, 1, 2, 2),  # blocks per elementwise chunk
    R_SPLITS=(1, 1, 2, 4),  # R input loaded in these block-granular pieces
    RAMP_N=3,               # first chunks run latency-optimal (DVE heavy)
    STAGE_V=12,             # of the 32 staging copies, this many go to DVE
    SUB_ENG="v",            # s6 -= ag engine: p=pool, v=vector
    W2_ENG="p",             # w2 = t - eyes engine
    AG_ENG="v",             # ag = alpha*G engine
    MUL_ENG="v",            # cross-product muls engine (v=DVE, p=Pool)
    SQ_ENG="a",             # squares engine steady state (a=ACT, p=Pool)
    ADD1_ENG="v",           # dots first add engine
    ADD2_ENG="v",           # dots second add engine
    ST_ENG="v",             # S^T PSUM->SBUF copy engine (a=ACT, v=DVE)
    TAIL_SPLIT=True,        # split last block's output DMA in two
    INV2F32=True,           # feed fp32 inv2 straight into the e-mul
    TR_DMA=False,           # transpose via DMA xbar instead of PE
    STAGGER=True,           # staggered sem reset in For_i timing loops
                            # (avoids the per-iteration all-engine barrier)
    IN_RING="s",            # input-load HWDGE ring (a=ACT, s=SP)
    RAMP_SQ="a",            # squares engine during ramp chunks
    OUT_RING="s",           # output-store HWDGE ring (a=ACT, s=SP); keeping
                            # the rings separate lets pass N+1 inputs dispatch
                            # while pass N outputs drain
    PSM_BUFS=3,             # [P,1024] f32 tiles = 2 banks each; psq takes 2
    STAGE_BUFS=3,
    ST_BUFS=3,
)


def _grid_q():
    ii, jj = np.meshgrid(np.arange(13), np.arange(13), indexing="ij")
    x = ((ii - 6) / 6.0).reshape(-1)
    y = ((jj - 6) / 6.0).reshape(-1)
    q = np.stack([x * x, y * y, np.ones(KG), 2 * x * y, 2 * x, 2 * y], axis=0)
    return q.astype(np.float16)  # [6, 169]


def make_q128():
    """[128, 676]: K=24 block-diag Q replicated at partition bases 0/32/64/96."""
    q6 = _grid_q()
    q24 = np.zeros((6 * GROUP, NMM), np.float16)
    for a in range(GROUP):
        q24[6 * a : 6 * a + 6, KG * a : KG * a + KG] = q6
    out = np.zeros((P, NMM), np.float16)
    for g in range(GPB):
        out[32 * g : 32 * g + 6 * GROUP, :] = q24
    return out


def _eng(nc, code):
    return {"v": nc.vector, "a": nc.scalar, "p": nc.gpsimd}[code]


def build_nc(reps: int = 1, loop_n: int = 0, **cfg_over):
    cfg = dict(CFG)
    cfg.update(cfg_over)
    nc = bacc.Bacc("TRN2", target_bir_lowering=False, debug=False,
                   num_devices=N_CORES)

    eyes_d = nc.declare_dram_parameter("eyes", [BC, 3], F32, isOutput=False)
    v_d = nc.declare_dram_parameter("v", [BC, 3], F32, isOutput=False)
    r_d = nc.declare_dram_parameter("R", [BC, 3, 3], F32, isOutput=False)
    t_d = nc.declare_dram_parameter("t", [BC, 3], F32, isOutput=False)
    a_d = nc.declare_dram_parameter("alpha", [BC], F32, isOutput=False)
    q_d = nc.declare_dram_parameter("q128", [P, NMM], F16, isOutput=False)
    id_d = nc.declare_dram_parameter("ident", [P, P], F16, isOutput=False)
    out_d = nc.declare_dram_parameter("out", [BC, KG], F16, isOutput=True)

    with tile.TileContext(nc) as tc:
        with ExitStack() as ctx:
            const = ctx.enter_context(tc.tile_pool(name="const", bufs=1))
            q_sb = const.tile([P, NMM], F16)
            id_sb = const.tile([P, P], F16)

            def load_consts():
                nc.sync.dma_start(q_sb[:], q_d.ap())
                if not cfg["TR_DMA"]:
                    nc.sync.dma_start(id_sb[:], id_d.ap())

            pools = dict(
                io=ctx.enter_context(tc.tile_pool(name="io", bufs=2)),
                scr=ctx.enter_context(tc.tile_pool(name="scr", bufs=2)),
                spool=ctx.enter_context(tc.tile_pool(name="sp", bufs=2)),
                stp=ctx.enter_context(
                    tc.tile_pool(name="st", bufs=cfg["ST_BUFS"])),
                stagep=ctx.enter_context(
                    tc.tile_pool(name="stage", bufs=cfg["STAGE_BUFS"])),
            )
            if not cfg["TR_DMA"]:
                pools["psq"] = ctx.enter_context(
                    tc.tile_pool(name="psq", bufs=2, space="PSUM"))
            pools["psm"] = ctx.enter_context(
                tc.tile_pool(name="psm", bufs=cfg["PSM_BUFS"], space="PSUM"))

            args = (nc, tc, pools, eyes_d, v_d, r_d, t_d, a_d, out_d,
                    q_sb, id_sb, cfg)
            if loop_n:
                load_consts()
                with tc.For_i(0, loop_n, 1,
                              staggered_reset=cfg["STAGGER"]):
                    for _ in range(reps):
                        _emit_one_pass(*args)
            else:
                for rep in range(reps):
                    _emit_one_pass(*args, load_consts if rep == 0 else None)

    nc.compile()
    return nc


def _emit_one_pass(nc, tc, pools, eyes_d, v_d, r_d, t_d, a_d, out_d,
                   q_sb, id_sb, cfg, load_consts=None):
    with ExitStack() as lpctx:
        lpctx.enter_context(
            nc.allow_low_precision(reason="fp16 kernel validated vs fp32 ref"))
        _emit_one_pass_lp(nc, tc, pools, eyes_d, v_d, r_d, t_d, a_d, out_d,
                          q_sb, id_sb, cfg, load_consts)


def _emit_one_pass_lp(nc, tc, pools, eyes_d, v_d, r_d, t_d, a_d, out_d,
                      q_sb, id_sb, cfg, load_consts):
    X = mybir.AxisListType.X
    ADD = mybir.AluOpType.add

    out_eng = nc.scalar if cfg["OUT_RING"] == "a" else nc.sync
    io = pools["io"]
    scr = pools["scr"]
    spool = pools["spool"]
    stp = pools["stp"]
    stagep = pools["stagep"]
    psm = pools["psm"]

    # DRAM views (per-partition contiguous)
    eyes_f = eyes_d.ap().rearrange("(p i) c -> p (i c)", p=P)
    v_f = v_d.ap().rearrange("(p i) c -> p (i c)", p=P)
    r_f = r_d.ap().rearrange("(p i) a b -> p (i a b)", p=P)
    t_f = t_d.ap().rearrange("(p i) k -> p (i k)", p=P)
    out_flat = out_d.ap().rearrange("(p i) k -> p (i k)", p=P)  # [P, NI*KG]

    # alpha: SWDGE cast load straight to fp16 (Pool is idle at pass start)
    a16 = io.tile([P, NI], F16, tag="alpha")
    nc.gpsimd.dma_start(a16[:], a_d.ap().rearrange("(p i) -> p i", p=P))

    # chunk table: (i0, ni, blk0, nb)
    assert sum(cfg["CHUNKS"]) == N_BLOCKS
    chunks = []
    b0 = 0
    for nb in cfg["CHUNKS"]:
        chunks.append((IPB * b0, IPB * nb, b0, nb))
        b0 += nb

    # input DMAs (HWDGE, fp32): first-chunk tensors first, consts after,
    # then the R remainder — minimizes time-to-first-compute
    eyes32 = io.tile([P, 3 * NI], F32, tag="eyes")
    v32 = io.tile([P, 3 * NI], F32, tag="v")
    t32 = io.tile([P, 3 * NI], F32, tag="t")
    r32 = io.tile([P, 9 * NI], F32, tag="r")
    assert sum(cfg["R_SPLITS"]) == N_BLOCKS
    in_eng = nc.scalar if cfg["IN_RING"] == "a" else nc.sync
    in_eng.dma_start(v32[:], v_f[:])
    lo, hi = 0, 9 * IPB * cfg["R_SPLITS"][0]
    in_eng.dma_start(r32[:, lo:hi], r_f[:, lo:hi])
    in_eng.dma_start(eyes32[:], eyes_f[:])
    in_eng.dma_start(t32[:], t_f[:])
    if load_consts is not None:
        load_consts()
    rb0 = cfg["R_SPLITS"][0]
    for rs in cfg["R_SPLITS"][1:]:
        lo, hi = 9 * IPB * rb0, 9 * IPB * (rb0 + rs)
        in_eng.dma_start(r32[:, lo:hi], r_f[:, lo:hi])
        rb0 += rs

    mm_idx = 0
    for ci, (i0, ni, blk0, nb) in enumerate(chunks):
        ngr = ni // GROUP

        w2 = scr.tile([P, 3 * ni], F32, tag=f"w2_{ci}")
        prod = scr.tile([P, 30 * ni], F16, tag=f"prod{ci}")
        dots9 = scr.tile([P, 9 * ni], F16, tag=f"d9_{ci}")
        nv2 = scr.tile([P, ni], F32, tag=f"nv2{ci}")
        inv2 = scr.tile([P, ni], F16, tag=f"inv2{ci}")
        e3 = scr.tile([P, 3 * ni], F16, tag=f"e3_{ci}")
        ag = scr.tile([P, 6 * ni], F16, tag=f"ag_{ci}")
        s_pad = spool.tile([P, GW * ngr], F16, tag=f"sp{ci}")

        v3 = v32[:, 3 * i0 : 3 * (i0 + ni)].rearrange("p (i c) -> p i c", c=3)
        t3 = t32[:, 3 * i0 : 3 * (i0 + ni)]
        ey3 = eyes32[:, 3 * i0 : 3 * (i0 + ni)]
        w23 = w2[:].rearrange("p (i c) -> p i c", c=3)
        rb = r32[:, 9 * i0 : 9 * (i0 + ni)].rearrange(
            "p (i a b) -> p b i a", a=3, b=3)
        pr = prod[:].rearrange("p (s i c) -> p s i c", s=10, c=3)

        ramp = ci < cfg["RAMP_N"]
        mul_e = "v" if ramp else cfg["MUL_ENG"]
        sq_e = cfg["RAMP_SQ"] if ramp else cfg["SQ_ENG"]
        add1_e = "v" if ramp else cfg["ADD1_ENG"]
        add2_e = "v" if ramp else cfg["ADD2_ENG"]
        ag_e = "v" if ramp else cfg["AG_ENG"]
        sub_e = "v" if ramp else cfg["SUB_ENG"]
        w2_e = "v" if ramp else cfg["W2_ENG"]

        # products (fp32 in -> fp16 out); v/R-only ones first so they can
        # start before eyes/t (and hence w2) are resident
        v_b2 = v3.unsqueeze(1).broadcast_to((P, 2, ni, 3))
        w_b2 = w23.unsqueeze(1).broadcast_to((P, 2, ni, 3))
        me = _eng(nc, mul_e)
        me.tensor_mul(pr[:, 1:3], v_b2, rb[:, 0:2])             # v.r0, v.r1
        me.tensor_mul(pr[:, 7], rb[:, 0], rb[:, 1])             # r0.r1
        if sq_e == "a":
            nc.scalar.square(pr[:, 0], v3)                      # v.v
            nc.scalar.square(pr[:, 4:6], rb[:, 0:2])            # r0.r0, r1.r1
        else:
            se = _eng(nc, sq_e)
            se.tensor_mul(pr[:, 0], v3, v3)
            se.tensor_mul(pr[:, 4:6], rb[:, 0:2], rb[:, 0:2])

        # w2 = t - eyes (fp32), then the w2-dependent products
        _eng(nc, w2_e).tensor_sub(w2[:], t3, ey3)
        me.tensor_mul(pr[:, 3], v3, w23)                        # v.w2
        me.tensor_mul(pr[:, 8:10], rb[:, 0:2], w_b2)            # r0.w2, r1.w2
        if sq_e == "a":
            nc.scalar.square(pr[:, 6], w23)                     # w2.w2
        else:
            se.tensor_mul(pr[:, 6], w23, w23)

        # dots: nv2 via reduce (fp32); the 9 dots via two strided adds
        # (d-major [9, ni] layout)
        d9 = dots9[:].rearrange("p (s i) -> p s i", s=9)
        nc.vector.tensor_reduce(nv2[:], pr[:, 0], axis=X, op=ADD)
        _eng(nc, add1_e).tensor_add(d9, pr[:, 1:10, :, 0], pr[:, 1:10, :, 1])
        _eng(nc, add2_e).tensor_add(d9, d9, pr[:, 1:10, :, 2])

        # inv2 = 1/nv2 (fp32; ||v||^2 >= 0.079 for these inputs)
        nc.vector.reciprocal(nv2[:], nv2[:])
        if not cfg["INV2F32"]:
            nc.vector.tensor_copy(inv2[:], nv2[:])

        # e = d * inv2 (fp16, [3, ni])
        e3v = e3[:].rearrange("p (s i) -> p s i", s=3)
        i_src = nv2 if cfg["INV2F32"] else inv2
        i_b3 = i_src[:].unsqueeze(1).broadcast_to((P, 3, ni))
        nc.vector.tensor_mul(e3v, d9[:, 0:3], i_b3)

        # raw-AP helpers over the padded sigma layout:
        # col(i=GROUP*gg+j, c) = GW*gg + 6*j + c
        s_h = s_pad[:].tensor
        s_o = s_pad[:].offset
        s_w = s_pad[:].ap[0][0]

        def sp_ap(c0, cn):
            return bass.AP(s_h, s_o + c0,
                           [[s_w, P], [GW, ngr], [6, GROUP], [1, cn]])

        d_h = dots9[:].tensor
        d_o = dots9[:].offset
        d_w = dots9[:].ap[0][0]

        def d_ap(s0, sn, s_stride=None):
            st = ni if s_stride is None else s_stride
            return bass.AP(d_h, d_o + s0 * ni,
                           [[d_w, P], [GROUP, ngr], [1, GROUP], [st, sn]])

        e_h = e3[:].tensor
        e_o = e3[:].offset
        e_w = e3[:].ap[0][0]

        def e_ap(s0, sn, s_stride=None):
            st = ni if s_stride is None else s_stride
            return bass.AP(e_h, e_o + s0 * ni,
                           [[e_w, P], [GROUP, ngr], [1, GROUP], [st, sn]])

        # s6 entries: diag c=0..2: e_c*d_c; c=3: e0*d1, c=4: e0*d2, c=5: e1*d2
        nc.vector.tensor_mul(sp_ap(0, 3), e_ap(0, 3), d_ap(0, 3))
        nc.vector.tensor_mul(sp_ap(3, 2), e_ap(0, 2, 0), d_ap(1, 2))
        nc.vector.tensor_mul(sp_ap(5, 1), e_ap(1, 1), d_ap(2, 1))

        # ag = alpha * G  (fp16, [6, ni] c-major)
        ag6 = ag[:].rearrange("p (s i) -> p s i", s=6)
        a_b6 = a16[:, i0 : i0 + ni].unsqueeze(1).broadcast_to((P, 6, ni))
        _eng(nc, ag_e).tensor_mul(ag6, d9[:, 3:9], a_b6)

        # s6 -= ag
        ag_h = ag[:].tensor
        ag_o = ag[:].offset
        ag_w = ag[:].ap[0][0]
        ag_p = bass.AP(ag_h, ag_o,
                       [[ag_w, P], [GROUP, ngr], [1, GROUP], [ni, 6]])
        _eng(nc, sub_e).tensor_sub(sp_ap(0, 6), sp_ap(0, 6), ag_p)

        # blocks: S^T transpose + 4 row-tiled matmuls + staged copies + DMA
        for lb in range(nb):
            b = blk0 + lb
            st_sb = stp.tile([P, P], F16, tag="stsb")
            s_slice = s_pad[:, P * lb : P * (lb + 1)]
            if cfg["TR_DMA"]:
                nc.scalar.dma_start_transpose(st_sb[:], s_slice)
            else:
                st_ps = pools["psq"].tile([P, P], F16, tag="stps")
                nc.tensor.transpose(st_ps[:], s_slice, id_sb[:])
                if cfg["ST_ENG"] == "a":
                    nc.scalar.copy(st_sb[:], st_ps[:])
                else:
                    nc.vector.tensor_copy(st_sb[:], st_ps[:])

            stage = stagep.tile([P, IPB * KG], F16, tag="stage")
            for g in range(GPB):
                o_ps = psm.tile([P, 1024], F32, tag="mmout")
                for h in range(2):
                    nc.tensor.matmul(
                        o_ps[:, 512 * h : 512 * h + NMH],
                        st_sb[32 * g : 32 * g + 6 * GROUP, :],
                        q_sb[32 * g : 32 * g + 6 * GROUP,
                             NMH * h : NMH * (h + 1)],
                        start=True,
                        stop=True,
                        tile_position=(32 * g, 0),
                    )
                if b == N_BLOCKS - 1:
                    eng = "v" if g % 2 == 0 else "a"  # tail: both engines
                else:
                    eng = ("v" if (mm_idx * cfg["STAGE_V"]) % 32
                           < cfg["STAGE_V"] else "a")
                mm_idx += 1
                o_h = o_ps[:].tensor
                o_o = o_ps[:].offset
                o_w = o_ps[:].ap[0][0]
                src = bass.AP(o_h, o_o, [[o_w, P], [512, 2], [1, NMH]])
                dst = stage[:, NMM * g : NMM * (g + 1)].rearrange(
                    "p (h k) -> p h k", h=2)
                if eng == "a":
                    nc.scalar.copy(dst, src)
                else:
                    nc.vector.tensor_copy(dst, src)
                if cfg["TAIL_SPLIT"] and b == N_BLOCKS - 1 and g == 1:
                    out_eng.dma_start(
                        out_flat[:, IPB * KG * b : IPB * KG * b + 2 * NMM],
                        stage[:, 0 : 2 * NMM],
                    )
            if cfg["TAIL_SPLIT"] and b == N_BLOCKS - 1:
                out_eng.dma_start(
                    out_flat[:, IPB * KG * b + 2 * NMM : IPB * KG * (b + 1)],
                    stage[:, 2 * NMM :],
                )
            else:
                out_eng.dma_start(
                    out_flat[:, IPB * KG * b : IPB * KG * (b + 1)], stage[:]
                )


_NC_CACHE = {}


def _get_nc(reps=1):
    if reps not in _NC_CACHE:
        _NC_CACHE[reps] = build_nc(reps)
    return _NC_CACHE[reps]


def make_in_maps(eyes, v, R, t, alpha):
    q128 = make_q128()
    ident = np.eye(P, dtype=np.float16)
    eyes = np.ascontiguousarray(eyes, np.float32).reshape(N_CORES, BC, 3)
    v = np.ascontiguousarray(v, np.float32).reshape(N_CORES, BC, 3)
    R = np.ascontiguousarray(R, np.float32).reshape(N_CORES, BC, 3, 3)
    t = np.ascontiguousarray(t, np.float32).reshape(N_CORES, BC, 3)
    alpha = np.ascontiguousarray(alpha, np.float32).reshape(N_CORES, BC)
    return [
        {
            "eyes": eyes[c], "v": v[c], "R": R[c], "t": t[c], "alpha": alpha[c],
            "q128": q128, "ident": ident,
        }
        for c in range(N_CORES)
    ]


def kernel(eyes, v, R, t, alpha):
    nc = _get_nc(1)
    in_maps = make_in_maps(eyes, v, R, t, alpha)
    res = run_bass_kernel_spmd(nc, in_maps, list(range(N_CORES)))
    out = np.concatenate([res.results[c]["out"] for c in range(N_CORES)], axis=0)
    return out.astype(np.float32)


# revision 29
# speedup vs baseline: 1.3395x; 1.3395x over previous
"""Trainium2 Bass kernel for ConeProjection (v3).

Math (per batch element b):
    W     = [R[:,0], R[:,1], t - eyes]          (3 rows)
    d_a   = v . W_a          (unnormalized)
    G_ac  = W_a . W_c
    inv2  = 1 / ||v||^2
    s     = (d d^T) * inv2 - alpha * G          (6 unique entries)
    out[k] = s . q[k],  q[k] = [x^2, y^2, 1, 2xy, 2x, 2y]  (169 grid pts)

Strategy: pure data-parallel over 8 NeuronCores (batch 131072 -> 16384/core).
Per core, partition p holds batch [p*NI, (p+1)*NI); within-partition index i.
Inputs load via HWDGE as fp32 (one DMA per tensor; R split for ramp); the
first multiply level reads fp32 and writes fp16. Sigma entries are written
directly into a transpose-ready padded layout (24 used / 32 cols per group of
4 i's), so PE transposes [128,128] slices straight out of the elementwise
output. Each block = 16 i's -> 1 transpose + 4 row-tiled matmul groups
(K=24, N=2x338 fp32 PSUM); per-group PSUM->SBUF copies alternate DVE/ACT
(the dominant engine cost), elementwise runs mostly on DVE/ACT with w2 on
Pool. One contiguous 692KB output DMA per block; the timing loop uses
staggered semaphore resets to soften the For_i all-engine barrier. Output
returns fp16, upcast to fp32 on the host. Measured: ~39.3us/pass loop metric,
~20us marginal (DMA roofline ~19us: 1.2MB in + 5.3MB out @ ~360GB/s/core).
"""

from contextlib import ExitStack, nullcontext

import numpy as np

import concourse.bass as bass
import concourse.bacc as bacc
import concourse.tile as tile
from concourse import mybir
from concourse.bass_utils import run_bass_kernel_spmd

N_CORES = 8
B = 131072
BC = B // N_CORES          # 16384 per core
P = 128                    # partitions
NI = BC // P               # 128 within-partition batch indices
KG = 169                   # grid points
F32 = mybir.dt.float32
F16 = mybir.dt.float16

GROUP = 4                  # i's per matmul group; K = 6*GROUP = 24
GPB = 4                    # groups per block (one [128,128] PE transpose)
IPB = GROUP * GPB          # 16 i per block
N_BLOCKS = NI // IPB       # 8
NMM = GROUP * KG           # 676 matmul free size
NMH = NMM // 2             # 338: matmul N split so fp32 out fits a PSUM bank
GW = 32                    # padded cols per group (24 used)

CFG = dict(
    CHUNKS=(1, 1, 1, 1, 2, 2),  # blocks per elementwise chunk
    R_SPLITS=(1, 7),        # R input loaded in these block-granular pieces
    RAMP_N=3,               # first chunks run latency-optimal (DVE heavy)
    STAGE_V=6,              # of the 32 staging copies, this many go to DVE
    RAMP_BLOCKS=0,          # early blocks alternate staging engines v/a
    SUB_ENG="v",            # s6 -= ag engine: p=pool, v=vector
    W2_ENG="p",             # w2 = t - eyes engine
    AG_ENG="v",             # ag = alpha*G engine
    MUL_ENG="v",            # cross-product muls engine (v=DVE, p=Pool)
    SQ_ENG="a",             # squares engine steady state (a=ACT, p=Pool)
    ADD1_ENG="p",           # dots first add engine
    ADD2_ENG="v",           # dots second add engine
    ST_ENG="v",             # S^T PSUM->SBUF copy engine (a=ACT, v=DVE)
    TAIL_SPLIT=True,        # split last block's output DMA in two
    INV2F32=True,           # feed fp32 inv2 straight into the e-mul
    TR_DMA=False,           # transpose via DMA xbar instead of PE
    STAGGER=True,           # staggered sem reset in For_i timing loops
                            # (avoids the per-iteration all-engine barrier)
    IN_RING="s",            # input-load HWDGE ring (a=ACT, s=SP)
    RAMP_SQ="a",            # squares engine during ramp chunks
    OUT_RING="s",           # output-store HWDGE ring (a=ACT, s=SP); keeping
                            # the rings separate lets pass N+1 inputs dispatch
                            # while pass N outputs drain
    PSM_BUFS=3,             # [P,1024] f32 tiles = 2 banks each; psq takes 2
    STAGE_BUFS=4,
    ST_BUFS=3,
)


def _grid_q():
    ii, jj = np.meshgrid(np.arange(13), np.arange(13), indexing="ij")
    x = ((ii - 6) / 6.0).reshape(-1)
    y = ((jj - 6) / 6.0).reshape(-1)
    q = np.stack([x * x, y * y, np.ones(KG), 2 * x * y, 2 * x, 2 * y], axis=0)
    return q.astype(np.float16)  # [6, 169]


def make_q128():
    """[128, 676]: K=24 block-diag Q replicated at partition bases 0/32/64/96."""
    q6 = _grid_q()
    q24 = np.zeros((6 * GROUP, NMM), np.float16)
    for a in range(GROUP):
        q24[6 * a : 6 * a + 6, KG * a : KG * a + KG] = q6
    out = np.zeros((P, NMM), np.float16)
    for g in range(GPB):
        out[32 * g : 32 * g + 6 * GROUP, :] = q24
    return out


def _eng(nc, code):
    return {"v": nc.vector, "a": nc.scalar, "p": nc.gpsimd}[code]


def build_nc(reps: int = 1, loop_n: int = 0, **cfg_over):
    cfg = dict(CFG)
    cfg.update(cfg_over)
    nc = bacc.Bacc("TRN2", target_bir_lowering=False, debug=False,
                   num_devices=N_CORES)

    eyes_d = nc.declare_dram_parameter("eyes", [BC, 3], F32, isOutput=False)
    v_d = nc.declare_dram_parameter("v", [BC, 3], F32, isOutput=False)
    r_d = nc.declare_dram_parameter("R", [BC, 3, 3], F32, isOutput=False)
    t_d = nc.declare_dram_parameter("t", [BC, 3], F32, isOutput=False)
    a_d = nc.declare_dram_parameter("alpha", [BC], F32, isOutput=False)
    q_d = nc.declare_dram_parameter("q128", [P, NMM], F16, isOutput=False)
    id_d = nc.declare_dram_parameter("ident", [P, P], F16, isOutput=False)
    out_d = nc.declare_dram_parameter("out", [BC, KG], F16, isOutput=True)

    with tile.TileContext(nc) as tc:
        with ExitStack() as ctx:
            const = ctx.enter_context(tc.tile_pool(name="const", bufs=1))
            q_sb = const.tile([P, NMM], F16)
            id_sb = const.tile([P, P], F16)

            def load_consts():
                nc.sync.dma_start(q_sb[:], q_d.ap())
                if not cfg["TR_DMA"]:
                    nc.sync.dma_start(id_sb[:], id_d.ap())

            pools = dict(
                io=ctx.enter_context(tc.tile_pool(name="io", bufs=2)),
                scr=ctx.enter_context(tc.tile_pool(name="scr", bufs=2)),
                spool=ctx.enter_context(tc.tile_pool(name="sp", bufs=2)),
                stp=ctx.enter_context(
                    tc.tile_pool(name="st", bufs=cfg["ST_BUFS"])),
                stagep=ctx.enter_context(
                    tc.tile_pool(name="stage", bufs=cfg["STAGE_BUFS"])),
            )
            if not cfg["TR_DMA"]:
                pools["psq"] = ctx.enter_context(
                    tc.tile_pool(name="psq", bufs=2, space="PSUM"))
            pools["psm"] = ctx.enter_context(
                tc.tile_pool(name="psm", bufs=cfg["PSM_BUFS"], space="PSUM"))

            args = (nc, tc, pools, eyes_d, v_d, r_d, t_d, a_d, out_d,
                    q_sb, id_sb, cfg)
            if loop_n:
                load_consts()
                with tc.For_i(0, loop_n, 1,
                              staggered_reset=cfg["STAGGER"]):
                    for _ in range(reps):
                        _emit_one_pass(*args)
            else:
                for rep in range(reps):
                    _emit_one_pass(*args, load_consts if rep == 0 else None)

    nc.compile()
    return nc


def _emit_one_pass(nc, tc, pools, eyes_d, v_d, r_d, t_d, a_d, out_d,
                   q_sb, id_sb, cfg, load_consts=None):
    with ExitStack() as lpctx:
        lpctx.enter_context(
            nc.allow_low_precision(reason="fp16 kernel validated vs fp32 ref"))
        _emit_one_pass_lp(nc, tc, pools, eyes_d, v_d, r_d, t_d, a_d, out_d,
                          q_sb, id_sb, cfg, load_consts)


def _emit_one_pass_lp(nc, tc, pools, eyes_d, v_d, r_d, t_d, a_d, out_d,
                      q_sb, id_sb, cfg, load_consts):
    X = mybir.AxisListType.X
    ADD = mybir.AluOpType.add

    out_eng = nc.scalar if cfg["OUT_RING"] == "a" else nc.sync
    io = pools["io"]
    scr = pools["scr"]
    spool = pools["spool"]
    stp = pools["stp"]
    stagep = pools["stagep"]
    psm = pools["psm"]

    # DRAM views (per-partition contiguous)
    eyes_f = eyes_d.ap().rearrange("(p i) c -> p (i c)", p=P)
    v_f = v_d.ap().rearrange("(p i) c -> p (i c)", p=P)
    r_f = r_d.ap().rearrange("(p i) a b -> p (i a b)", p=P)
    t_f = t_d.ap().rearrange("(p i) k -> p (i k)", p=P)
    out_flat = out_d.ap().rearrange("(p i) k -> p (i k)", p=P)  # [P, NI*KG]

    # alpha: SWDGE cast load straight to fp16 (Pool is idle at pass start)
    a16 = io.tile([P, NI], F16, tag="alpha")
    nc.gpsimd.dma_start(a16[:], a_d.ap().rearrange("(p i) -> p i", p=P))

    # chunk table: (i0, ni, blk0, nb)
    assert sum(cfg["CHUNKS"]) == N_BLOCKS
    chunks = []
    b0 = 0
    for nb in cfg["CHUNKS"]:
        chunks.append((IPB * b0, IPB * nb, b0, nb))
        b0 += nb

    # input DMAs (HWDGE, fp32): first-chunk tensors first, consts after,
    # then the R remainder — minimizes time-to-first-compute
    eyes32 = io.tile([P, 3 * NI], F32, tag="eyes")
    v32 = io.tile([P, 3 * NI], F32, tag="v")
    t32 = io.tile([P, 3 * NI], F32, tag="t")
    r32 = io.tile([P, 9 * NI], F32, tag="r")
    assert sum(cfg["R_SPLITS"]) == N_BLOCKS
    in_eng = nc.scalar if cfg["IN_RING"] == "a" else nc.sync
    in_eng.dma_start(v32[:], v_f[:])
    lo, hi = 0, 9 * IPB * cfg["R_SPLITS"][0]
    in_eng.dma_start(r32[:, lo:hi], r_f[:, lo:hi])
    in_eng.dma_start(eyes32[:], eyes_f[:])
    in_eng.dma_start(t32[:], t_f[:])
    if load_consts is not None:
        load_consts()
    rb0 = cfg["R_SPLITS"][0]
    for rs in cfg["R_SPLITS"][1:]:
        lo, hi = 9 * IPB * rb0, 9 * IPB * (rb0 + rs)
        in_eng.dma_start(r32[:, lo:hi], r_f[:, lo:hi])
        rb0 += rs

    mm_idx = 0
    for ci, (i0, ni, blk0, nb) in enumerate(chunks):
        ngr = ni // GROUP

        w2 = scr.tile([P, 3 * ni], F32, tag=f"w2_{ci}")
        prod = scr.tile([P, 30 * ni], F16, tag=f"prod{ci}")
        dots9 = scr.tile([P, 9 * ni], F16, tag=f"d9_{ci}")
        nv2 = scr.tile([P, ni], F32, tag=f"nv2{ci}")
        inv2 = scr.tile([P, ni], F16, tag=f"inv2{ci}")
        e3 = scr.tile([P, 3 * ni], F16, tag=f"e3_{ci}")
        ag = scr.tile([P, 6 * ni], F16, tag=f"ag_{ci}")
        s_pad = spool.tile([P, GW * ngr], F16, tag=f"sp{ci}")

        v3 = v32[:, 3 * i0 : 3 * (i0 + ni)].rearrange("p (i c) -> p i c", c=3)
        t3 = t32[:, 3 * i0 : 3 * (i0 + ni)]
        ey3 = eyes32[:, 3 * i0 : 3 * (i0 + ni)]
        w23 = w2[:].rearrange("p (i c) -> p i c", c=3)
        rb = r32[:, 9 * i0 : 9 * (i0 + ni)].rearrange(
            "p (i a b) -> p b i a", a=3, b=3)
        pr = prod[:].rearrange("p (s i c) -> p s i c", s=10, c=3)

        ramp = ci < cfg["RAMP_N"]
        mul_e = "v" if ramp else cfg["MUL_ENG"]
        sq_e = cfg["RAMP_SQ"] if ramp else cfg["SQ_ENG"]
        add1_e = "v" if ramp else cfg["ADD1_ENG"]
        add2_e = "v" if ramp else cfg["ADD2_ENG"]
        ag_e = "v" if ramp else cfg["AG_ENG"]
        sub_e = "v" if ramp else cfg["SUB_ENG"]
        w2_e = "v" if ramp else cfg["W2_ENG"]

        # products (fp32 in -> fp16 out); v/R-only ones first so they can
        # start before eyes/t (and hence w2) are resident
        v_b2 = v3.unsqueeze(1).broadcast_to((P, 2, ni, 3))
        w_b2 = w23.unsqueeze(1).broadcast_to((P, 2, ni, 3))
        me = _eng(nc, mul_e)
        me.tensor_mul(pr[:, 1:3], v_b2, rb[:, 0:2])             # v.r0, v.r1
        me.tensor_mul(pr[:, 7], rb[:, 0], rb[:, 1])             # r0.r1
        if sq_e == "a":
            nc.scalar.square(pr[:, 0], v3)                      # v.v
            nc.scalar.square(pr[:, 4:6], rb[:, 0:2])            # r0.r0, r1.r1
        else:
            se = _eng(nc, sq_e)
            se.tensor_mul(pr[:, 0], v3, v3)
            se.tensor_mul(pr[:, 4:6], rb[:, 0:2], rb[:, 0:2])

        # w2 = t - eyes (fp32), then the w2-dependent products
        _eng(nc, w2_e).tensor_sub(w2[:], t3, ey3)
        me.tensor_mul(pr[:, 3], v3, w23)                        # v.w2
        me.tensor_mul(pr[:, 8:10], rb[:, 0:2], w_b2)            # r0.w2, r1.w2
        if sq_e == "a":
            nc.scalar.square(pr[:, 6], w23)                     # w2.w2
        else:
            se.tensor_mul(pr[:, 6], w23, w23)

        # dots: nv2 via reduce (fp32); the 9 dots via two strided adds
        # (d-major [9, ni] layout)
        d9 = dots9[:].rearrange("p (s i) -> p s i", s=9)
        nc.vector.tensor_reduce(nv2[:], pr[:, 0], axis=X, op=ADD)
        _eng(nc, add1_e).tensor_add(d9, pr[:, 1:10, :, 0], pr[:, 1:10, :, 1])
        _eng(nc, add2_e).tensor_add(d9, d9, pr[:, 1:10, :, 2])

        # inv2 = 1/nv2 (fp32; ||v||^2 >= 0.079 for these inputs)
        nc.vector.reciprocal(nv2[:], nv2[:])
        if not cfg["INV2F32"]:
            nc.vector.tensor_copy(inv2[:], nv2[:])

        # e = d * inv2 (fp16, [3, ni])
        e3v = e3[:].rearrange("p (s i) -> p s i", s=3)
        i_src = nv2 if cfg["INV2F32"] else inv2
        i_b3 = i_src[:].unsqueeze(1).broadcast_to((P, 3, ni))
        nc.vector.tensor_mul(e3v, d9[:, 0:3], i_b3)

        # raw-AP helpers over the padded sigma layout:
        # col(i=GROUP*gg+j, c) = GW*gg + 6*j + c
        s_h = s_pad[:].tensor
        s_o = s_pad[:].offset
        s_w = s_pad[:].ap[0][0]

        def sp_ap(c0, cn):
            return bass.AP(s_h, s_o + c0,
                           [[s_w, P], [GW, ngr], [6, GROUP], [1, cn]])

        d_h = dots9[:].tensor
        d_o = dots9[:].offset
        d_w = dots9[:].ap[0][0]

        def d_ap(s0, sn, s_stride=None):
            st = ni if s_stride is None else s_stride
            return bass.AP(d_h, d_o + s0 * ni,
                           [[d_w, P], [GROUP, ngr], [1, GROUP], [st, sn]])

        e_h = e3[:].tensor
        e_o = e3[:].offset
        e_w = e3[:].ap[0][0]

        def e_ap(s0, sn, s_stride=None):
            st = ni if s_stride is None else s_stride
            return bass.AP(e_h, e_o + s0 * ni,
                           [[e_w, P], [GROUP, ngr], [1, GROUP], [st, sn]])

        # s6 entries: diag c=0..2: e_c*d_c; c=3: e0*d1, c=4: e0*d2, c=5: e1*d2
        nc.vector.tensor_mul(sp_ap(0, 3), e_ap(0, 3), d_ap(0, 3))
        nc.vector.tensor_mul(sp_ap(3, 2), e_ap(0, 2, 0), d_ap(1, 2))
        nc.vector.tensor_mul(sp_ap(5, 1), e_ap(1, 1), d_ap(2, 1))

        # ag = alpha * G  (fp16, [6, ni] c-major)
        ag6 = ag[:].rearrange("p (s i) -> p s i", s=6)
        a_b6 = a16[:, i0 : i0 + ni].unsqueeze(1).broadcast_to((P, 6, ni))
        _eng(nc, ag_e).tensor_mul(ag6, d9[:, 3:9], a_b6)

        # s6 -= ag
        ag_h = ag[:].tensor
        ag_o = ag[:].offset
        ag_w = ag[:].ap[0][0]
        ag_p = bass.AP(ag_h, ag_o,
                       [[ag_w, P], [GROUP, ngr], [1, GROUP], [ni, 6]])
        _eng(nc, sub_e).tensor_sub(sp_ap(0, 6), sp_ap(0, 6), ag_p)

        # blocks: S^T transpose + 4 row-tiled matmuls + staged copies + DMA
        for lb in range(nb):
            b = blk0 + lb
            st_sb = stp.tile([P, P], F16, tag="stsb")
            s_slice = s_pad[:, P * lb : P * (lb + 1)]
            if cfg["TR_DMA"]:
                nc.scalar.dma_start_transpose(st_sb[:], s_slice)
            else:
                st_ps = pools["psq"].tile([P, P], F16, tag="stps")
                nc.tensor.transpose(st_ps[:], s_slice, id_sb[:])
                if cfg["ST_ENG"] == "a":
                    nc.scalar.copy(st_sb[:], st_ps[:])
                else:
                    nc.vector.tensor_copy(st_sb[:], st_ps[:])

            stage = stagep.tile([P, IPB * KG], F16, tag="stage")
            for g in range(GPB):
                o_ps = psm.tile([P, 1024], F32, tag="mmout")
                for h in range(2):
                    nc.tensor.matmul(
                        o_ps[:, 512 * h : 512 * h + NMH],
                        st_sb[32 * g : 32 * g + 6 * GROUP, :],
                        q_sb[32 * g : 32 * g + 6 * GROUP,
                             NMH * h : NMH * (h + 1)],
                        start=True,
                        stop=True,
                        tile_position=(32 * g, 0),
                    )
                if b == N_BLOCKS - 1 or b < cfg["RAMP_BLOCKS"]:
                    eng = "v" if g % 2 == 0 else "a"  # ramp/tail: both engines
                else:
                    eng = ("v" if (mm_idx * cfg["STAGE_V"]) % 32
                           < cfg["STAGE_V"] else "a")
                mm_idx += 1
                o_h = o_ps[:].tensor
                o_o = o_ps[:].offset
                o_w = o_ps[:].ap[0][0]
                src = bass.AP(o_h, o_o, [[o_w, P], [512, 2], [1, NMH]])
                dst = stage[:, NMM * g : NMM * (g + 1)].rearrange(
                    "p (h k) -> p h k", h=2)
                if eng == "a":
                    nc.scalar.copy(dst, src)
                else:
                    nc.vector.tensor_copy(dst, src)
                if cfg["TAIL_SPLIT"] and b == N_BLOCKS - 1 and g == 1:
                    out_eng.dma_start(
                        out_flat[:, IPB * KG * b : IPB * KG * b + 2 * NMM],
                        stage[:, 0 : 2 * NMM],
                    )
            if cfg["TAIL_SPLIT"] and b == N_BLOCKS - 1:
                out_eng.dma_start(
                    out_flat[:, IPB * KG * b + 2 * NMM : IPB * KG * (b + 1)],
                    stage[:, 2 * NMM :],
                )
            else:
                out_eng.dma_start(
                    out_flat[:, IPB * KG * b : IPB * KG * (b + 1)], stage[:]
                )


_NC_CACHE = {}


def _get_nc(reps=1):
    if reps not in _NC_CACHE:
        _NC_CACHE[reps] = build_nc(reps)
    return _NC_CACHE[reps]


def make_in_maps(eyes, v, R, t, alpha):
    q128 = make_q128()
    ident = np.eye(P, dtype=np.float16)
    eyes = np.ascontiguousarray(eyes, np.float32).reshape(N_CORES, BC, 3)
    v = np.ascontiguousarray(v, np.float32).reshape(N_CORES, BC, 3)
    R = np.ascontiguousarray(R, np.float32).reshape(N_CORES, BC, 3, 3)
    t = np.ascontiguousarray(t, np.float32).reshape(N_CORES, BC, 3)
    alpha = np.ascontiguousarray(alpha, np.float32).reshape(N_CORES, BC)
    return [
        {
            "eyes": eyes[c], "v": v[c], "R": R[c], "t": t[c], "alpha": alpha[c],
            "q128": q128, "ident": ident,
        }
        for c in range(N_CORES)
    ]


def kernel(eyes, v, R, t, alpha):
    nc = _get_nc(1)
    in_maps = make_in_maps(eyes, v, R, t, alpha)
    res = run_bass_kernel_spmd(nc, in_maps, list(range(N_CORES)))
    out = np.concatenate([res.results[c]["out"] for c in range(N_CORES)], axis=0)
    return out.astype(np.float32)


# revision 34
# speedup vs baseline: 1.3654x; 1.0194x over previous
"""Trainium2 Bass kernel for ConeProjection (v3).

Math (per batch element b):
    W     = [R[:,0], R[:,1], t - eyes]          (3 rows)
    d_a   = v . W_a          (unnormalized)
    G_ac  = W_a . W_c
    inv2  = 1 / ||v||^2
    s     = (d d^T) * inv2 - alpha * G          (6 unique entries)
    out[k] = s . q[k],  q[k] = [x^2, y^2, 1, 2xy, 2x, 2y]  (169 grid pts)

Strategy: pure data-parallel over 8 NeuronCores (batch 131072 -> 16384/core).
Per core, partition p holds batch [p*NI, (p+1)*NI); within-partition index i.
Inputs load via HWDGE as fp32 (one DMA per tensor; R split for ramp); the
first multiply level reads fp32 and writes fp16. Sigma entries are written
directly into a transpose-ready padded layout (24 used / 32 cols per group of
4 i's), so PE transposes [128,128] slices straight out of the elementwise
output. Each block = 16 i's -> 1 transpose + 4 row-tiled matmul groups
(K=24, N=2x338 fp32 PSUM); per-group PSUM->SBUF copies alternate DVE/ACT
(the dominant engine cost), elementwise runs mostly on DVE/ACT with w2 on
Pool. One contiguous 692KB output DMA per block; the timing loop uses
staggered semaphore resets to soften the For_i all-engine barrier. Output
returns fp16, upcast to fp32 on the host. Measured: 35.1us/pass loop metric
(baseline 63.6us recorded / 49.4us same-day), ~20us marginal-pass = the DMA
roofline (1.2MB in + 5.3MB out @ ~360GB/s/core ~= 19us).
"""

from contextlib import ExitStack, nullcontext

import numpy as np

import concourse.bass as bass
import concourse.bacc as bacc
import concourse.tile as tile
from concourse import mybir
from concourse.bass_utils import run_bass_kernel_spmd

N_CORES = 8
B = 131072
BC = B // N_CORES          # 16384 per core
P = 128                    # partitions
NI = BC // P               # 128 within-partition batch indices
KG = 169                   # grid points
F32 = mybir.dt.float32
F16 = mybir.dt.float16

GROUP = 4                  # i's per matmul group; K = 6*GROUP = 24
GPB = 4                    # groups per block (one [128,128] PE transpose)
IPB = GROUP * GPB          # 16 i per block
N_BLOCKS = NI // IPB       # 8
NMM = GROUP * KG           # 676 matmul free size
NMH = NMM // 2             # 338: matmul N split so fp32 out fits a PSUM bank
GW = 32                    # padded cols per group (24 used)

CFG = dict(
    CHUNKS=(1, 1, 1, 1, 2, 2),  # blocks per elementwise chunk
    R_SPLITS=(1, 7),        # R input loaded in these block-granular pieces
    RAMP_N=3,               # first chunks run latency-optimal (DVE heavy)
    STAGE_V=6,              # of the 32 staging copies, this many go to DVE
    RAMP_BLOCKS=0,          # early blocks alternate staging engines v/a
    SUB_ENG="v",            # s6 -= ag engine: p=pool, v=vector
    W2_ENG="p",             # w2 = t - eyes engine
    AG_ENG="v",             # ag = alpha*G engine
    MUL_ENG="v",            # cross-product muls engine (v=DVE, p=Pool)
    SQ_ENG="a",             # squares engine steady state (a=ACT, p=Pool)
    ADD1_ENG="p",           # dots first add engine
    ADD2_ENG="v",           # dots second add engine
    ST_ENG="v",             # S^T PSUM->SBUF copy engine (a=ACT, v=DVE)
    TAIL_SPLIT=True,        # split last block's output DMA in two
    INV2F32=True,           # feed fp32 inv2 straight into the e-mul
    TR_DMA=False,           # transpose via DMA xbar instead of PE
    STAGGER=True,           # staggered sem reset in For_i timing loops
                            # (avoids the per-iteration all-engine barrier)
    IN_RING="s",            # input-load HWDGE ring (a=ACT, s=SP)
    RAMP_SQ="a",            # squares engine during ramp chunks
    OUT_RING="s",           # output-store HWDGE ring (a=ACT, s=SP); keeping
                            # the rings separate lets pass N+1 inputs dispatch
                            # while pass N outputs drain
    HALF0=False,            # split chunk0 into two 2-group halves (earlier
                            # first transpose/matmul/DMA at slight op overhead)
    PE_WARM=0,              # dummy matmuls at pass start to hold HAM at 2.4GHz
    PE_WARM_F32=0,          # paced fp32 warm matmuls on the R tile: each runs
                            # ~0.6-1.1us, keeping PE continuously busy from R0
                            # arrival until real work so blocks start at 2.4GHz
    EYEST_SWDGE=False,      # load eyes/t via Pool SWDGE (parallel to HWDGE)
    PSM_BUFS=3,             # [P,1024] f32 tiles = 2 banks each; psq takes 2
    STAGE_BUFS=4,
    ST_BUFS=3,
)


def _grid_q():
    ii, jj = np.meshgrid(np.arange(13), np.arange(13), indexing="ij")
    x = ((ii - 6) / 6.0).reshape(-1)
    y = ((jj - 6) / 6.0).reshape(-1)
    q = np.stack([x * x, y * y, np.ones(KG), 2 * x * y, 2 * x, 2 * y], axis=0)
    return q.astype(np.float16)  # [6, 169]


def make_q128():
    """[128, 676]: K=24 block-diag Q replicated at partition bases 0/32/64/96."""
    q6 = _grid_q()
    q24 = np.zeros((6 * GROUP, NMM), np.float16)
    for a in range(GROUP):
        q24[6 * a : 6 * a + 6, KG * a : KG * a + KG] = q6
    out = np.zeros((P, NMM), np.float16)
    for g in range(GPB):
        out[32 * g : 32 * g + 6 * GROUP, :] = q24
    return out


def _eng(nc, code):
    return {"v": nc.vector, "a": nc.scalar, "p": nc.gpsimd}[code]


def build_nc(reps: int = 1, loop_n: int = 0, **cfg_over):
    cfg = dict(CFG)
    cfg.update(cfg_over)
    nc = bacc.Bacc("TRN2", target_bir_lowering=False, debug=False,
                   num_devices=N_CORES)

    eyes_d = nc.declare_dram_parameter("eyes", [BC, 3], F32, isOutput=False)
    v_d = nc.declare_dram_parameter("v", [BC, 3], F32, isOutput=False)
    r_d = nc.declare_dram_parameter("R", [BC, 3, 3], F32, isOutput=False)
    t_d = nc.declare_dram_parameter("t", [BC, 3], F32, isOutput=False)
    a_d = nc.declare_dram_parameter("alpha", [BC], F32, isOutput=False)
    q_d = nc.declare_dram_parameter("q128", [P, NMM], F16, isOutput=False)
    id_d = nc.declare_dram_parameter("ident", [P, P], F16, isOutput=False)
    out_d = nc.declare_dram_parameter("out", [BC, KG], F16, isOutput=True)

    with tile.TileContext(nc) as tc:
        with ExitStack() as ctx:
            const = ctx.enter_context(tc.tile_pool(name="const", bufs=1))
            q_sb = const.tile([P, NMM], F16)
            id_sb = const.tile([P, P], F16)

            def load_consts():
                nc.sync.dma_start(q_sb[:], q_d.ap())
                if not cfg["TR_DMA"]:
                    nc.sync.dma_start(id_sb[:], id_d.ap())

            pools = dict(
                io=ctx.enter_context(tc.tile_pool(name="io", bufs=2)),
                scr=ctx.enter_context(tc.tile_pool(name="scr", bufs=2)),
                spool=ctx.enter_context(tc.tile_pool(name="sp", bufs=2)),
                stp=ctx.enter_context(
                    tc.tile_pool(name="st", bufs=cfg["ST_BUFS"])),
                stagep=ctx.enter_context(
                    tc.tile_pool(name="stage", bufs=cfg["STAGE_BUFS"])),
            )
            if not cfg["TR_DMA"]:
                pools["psq"] = ctx.enter_context(
                    tc.tile_pool(name="psq", bufs=2, space="PSUM"))
            pools["psm"] = ctx.enter_context(
                tc.tile_pool(name="psm", bufs=cfg["PSM_BUFS"], space="PSUM"))

            args = (nc, tc, pools, eyes_d, v_d, r_d, t_d, a_d, out_d,
                    q_sb, id_sb, cfg)
            if loop_n:
                load_consts()
                with tc.For_i(0, loop_n, 1,
                              staggered_reset=cfg["STAGGER"]):
                    for _ in range(reps):
                        _emit_one_pass(*args)
            else:
                for rep in range(reps):
                    _emit_one_pass(*args, load_consts if rep == 0 else None)

    nc.compile()
    return nc


def _emit_one_pass(nc, tc, pools, eyes_d, v_d, r_d, t_d, a_d, out_d,
                   q_sb, id_sb, cfg, load_consts=None):
    with ExitStack() as lpctx:
        lpctx.enter_context(
            nc.allow_low_precision(reason="fp16 kernel validated vs fp32 ref"))
        _emit_one_pass_lp(nc, tc, pools, eyes_d, v_d, r_d, t_d, a_d, out_d,
                          q_sb, id_sb, cfg, load_consts)


def _emit_one_pass_lp(nc, tc, pools, eyes_d, v_d, r_d, t_d, a_d, out_d,
                      q_sb, id_sb, cfg, load_consts):
    X = mybir.AxisListType.X
    ADD = mybir.AluOpType.add

    out_eng = nc.scalar if cfg["OUT_RING"] == "a" else nc.sync
    io = pools["io"]
    scr = pools["scr"]
    spool = pools["spool"]
    stp = pools["stp"]
    stagep = pools["stagep"]
    psm = pools["psm"]

    # DRAM views (per-partition contiguous)
    eyes_f = eyes_d.ap().rearrange("(p i) c -> p (i c)", p=P)
    v_f = v_d.ap().rearrange("(p i) c -> p (i c)", p=P)
    r_f = r_d.ap().rearrange("(p i) a b -> p (i a b)", p=P)
    t_f = t_d.ap().rearrange("(p i) k -> p (i k)", p=P)
    out_flat = out_d.ap().rearrange("(p i) k -> p (i k)", p=P)  # [P, NI*KG]

    # alpha: SWDGE cast load straight to fp16 (Pool is idle at pass start)
    a16 = io.tile([P, NI], F16, tag="alpha")
    nc.gpsimd.dma_start(a16[:], a_d.ap().rearrange("(p i) -> p i", p=P))

    # chunk table: (i0, ni, blk0, nb, half)
    assert sum(cfg["CHUNKS"]) == N_BLOCKS
    chunks = []
    b0 = 0
    for nb in cfg["CHUNKS"]:
        chunks.append((IPB * b0, IPB * nb, b0, nb, None))
        b0 += nb
    if cfg["HALF0"] and chunks[0][3] == 1:
        half_ni = IPB // 2
        chunks[0:1] = [(0, half_ni, 0, 0, "A"), (half_ni, half_ni, 0, 0, "B")]

    # input DMAs (HWDGE, fp32): first-chunk tensors first, consts after,
    # then the R remainder — minimizes time-to-first-compute
    eyes32 = io.tile([P, 3 * NI], F32, tag="eyes")
    v32 = io.tile([P, 3 * NI], F32, tag="v")
    t32 = io.tile([P, 3 * NI], F32, tag="t")
    r32 = io.tile([P, 9 * NI], F32, tag="r")
    assert sum(cfg["R_SPLITS"]) == N_BLOCKS
    in_eng = nc.scalar if cfg["IN_RING"] == "a" else nc.sync
    eyest_eng = nc.gpsimd if cfg["EYEST_SWDGE"] else in_eng
    in_eng.dma_start(v32[:], v_f[:])
    lo, hi = 0, 9 * IPB * cfg["R_SPLITS"][0]
    in_eng.dma_start(r32[:, lo:hi], r_f[:, lo:hi])
    eyest_eng.dma_start(eyes32[:], eyes_f[:])
    eyest_eng.dma_start(t32[:], t_f[:])
    if load_consts is not None:
        load_consts()
    for wi in range(cfg["PE_WARM_F32"]):
        warm = psm.tile([P, 1024], F32, tag="mmout")
        nc.tensor.matmul(
            warm[:, 0:NMH], r32[0:24, 0:P], r32[0:24, 0:NMH],
            start=True, stop=True, tile_position=(0, 0),
        )

    rb0 = cfg["R_SPLITS"][0]
    for rs in cfg["R_SPLITS"][1:]:
        lo, hi = 9 * IPB * rb0, 9 * IPB * (rb0 + rs)
        in_eng.dma_start(r32[:, lo:hi], r_f[:, lo:hi])
        rb0 += rs

    # hold the PE HAM at full clock through the ramp: the PE idles during
    # the input phase each iteration, and a >3.4us idle re-throttles it
    for wi in range(cfg["PE_WARM"]):
        warm = psm.tile([P, 1024], F32, tag="mmout")
        nc.tensor.matmul(
            warm[:, 0:NMH], q_sb[0:24, 0:P], q_sb[0:24, 0:NMH],
            start=True, stop=True, tile_position=(0, 0),
        )

    mm_idx = 0
    half_stage = [None]
    for ci, (i0, ni, blk0, nb, half) in enumerate(chunks):
        ngr = ni // GROUP

        w2 = scr.tile([P, 3 * ni], F32, tag=f"w2_{ci}")
        prod = scr.tile([P, 30 * ni], F16, tag=f"prod{ci}")
        dots9 = scr.tile([P, 9 * ni], F16, tag=f"d9_{ci}")
        nv2 = scr.tile([P, ni], F32, tag=f"nv2{ci}")
        inv2 = scr.tile([P, ni], F16, tag=f"inv2{ci}")
        e3 = scr.tile([P, 3 * ni], F16, tag=f"e3_{ci}")
        ag = scr.tile([P, 6 * ni], F16, tag=f"ag_{ci}")
        s_pad = spool.tile([P, GW * ngr], F16, tag=f"sp{ci}")

        v3 = v32[:, 3 * i0 : 3 * (i0 + ni)].rearrange("p (i c) -> p i c", c=3)
        t3 = t32[:, 3 * i0 : 3 * (i0 + ni)]
        ey3 = eyes32[:, 3 * i0 : 3 * (i0 + ni)]
        w23 = w2[:].rearrange("p (i c) -> p i c", c=3)
        rb = r32[:, 9 * i0 : 9 * (i0 + ni)].rearrange(
            "p (i a b) -> p b i a", a=3, b=3)
        pr = prod[:].rearrange("p (s i c) -> p s i c", s=10, c=3)

        ramp = ci < cfg["RAMP_N"]
        mul_e = "v" if ramp else cfg["MUL_ENG"]
        sq_e = cfg["RAMP_SQ"] if ramp else cfg["SQ_ENG"]
        add1_e = "v" if ramp else cfg["ADD1_ENG"]
        add2_e = "v" if ramp else cfg["ADD2_ENG"]
        ag_e = "v" if ramp else cfg["AG_ENG"]
        sub_e = "v" if ramp else cfg["SUB_ENG"]
        w2_e = "v" if ramp else cfg["W2_ENG"]

        # products (fp32 in -> fp16 out); v/R-only ones first so they can
        # start before eyes/t (and hence w2) are resident
        v_b2 = v3.unsqueeze(1).broadcast_to((P, 2, ni, 3))
        w_b2 = w23.unsqueeze(1).broadcast_to((P, 2, ni, 3))
        me = _eng(nc, mul_e)
        me.tensor_mul(pr[:, 1:3], v_b2, rb[:, 0:2])             # v.r0, v.r1
        me.tensor_mul(pr[:, 7], rb[:, 0], rb[:, 1])             # r0.r1
        if sq_e == "a":
            nc.scalar.square(pr[:, 0], v3)                      # v.v
            nc.scalar.square(pr[:, 4:6], rb[:, 0:2])            # r0.r0, r1.r1
        else:
            se = _eng(nc, sq_e)
            se.tensor_mul(pr[:, 0], v3, v3)
            se.tensor_mul(pr[:, 4:6], rb[:, 0:2], rb[:, 0:2])

        # w2 = t - eyes (fp32), then the w2-dependent products
        _eng(nc, w2_e).tensor_sub(w2[:], t3, ey3)
        me.tensor_mul(pr[:, 3], v3, w23)                        # v.w2
        me.tensor_mul(pr[:, 8:10], rb[:, 0:2], w_b2)            # r0.w2, r1.w2
        if sq_e == "a":
            nc.scalar.square(pr[:, 6], w23)                     # w2.w2
        else:
            se.tensor_mul(pr[:, 6], w23, w23)

        # dots: nv2 via reduce (fp32); the 9 dots via two strided adds
        # (d-major [9, ni] layout)
        d9 = dots9[:].rearrange("p (s i) -> p s i", s=9)
        nc.vector.tensor_reduce(nv2[:], pr[:, 0], axis=X, op=ADD)
        _eng(nc, add1_e).tensor_add(d9, pr[:, 1:10, :, 0], pr[:, 1:10, :, 1])
        _eng(nc, add2_e).tensor_add(d9, d9, pr[:, 1:10, :, 2])

        # inv2 = 1/nv2 (fp32; ||v||^2 >= 0.079 for these inputs)
        nc.vector.reciprocal(nv2[:], nv2[:])
        if not cfg["INV2F32"]:
            nc.vector.tensor_copy(inv2[:], nv2[:])

        # e = d * inv2 (fp16, [3, ni])
        e3v = e3[:].rearrange("p (s i) -> p s i", s=3)
        i_src = nv2 if cfg["INV2F32"] else inv2
        i_b3 = i_src[:].unsqueeze(1).broadcast_to((P, 3, ni))
        nc.vector.tensor_mul(e3v, d9[:, 0:3], i_b3)

        # raw-AP helpers over the padded sigma layout:
        # col(i=GROUP*gg+j, c) = GW*gg + 6*j + c
        s_h = s_pad[:].tensor
        s_o = s_pad[:].offset
        s_w = s_pad[:].ap[0][0]

        def sp_ap(c0, cn):
            return bass.AP(s_h, s_o + c0,
                           [[s_w, P], [GW, ngr], [6, GROUP], [1, cn]])

        d_h = dots9[:].tensor
        d_o = dots9[:].offset
        d_w = dots9[:].ap[0][0]

        def d_ap(s0, sn, s_stride=None):
            st = ni if s_stride is None else s_stride
            return bass.AP(d_h, d_o + s0 * ni,
                           [[d_w, P], [GROUP, ngr], [1, GROUP], [st, sn]])

        e_h = e3[:].tensor
        e_o = e3[:].offset
        e_w = e3[:].ap[0][0]

        def e_ap(s0, sn, s_stride=None):
            st = ni if s_stride is None else s_stride
            return bass.AP(e_h, e_o + s0 * ni,
                           [[e_w, P], [GROUP, ngr], [1, GROUP], [st, sn]])

        # s6 entries: diag c=0..2: e_c*d_c; c=3: e0*d1, c=4: e0*d2, c=5: e1*d2
        nc.vector.tensor_mul(sp_ap(0, 3), e_ap(0, 3), d_ap(0, 3))
        nc.vector.tensor_mul(sp_ap(3, 2), e_ap(0, 2, 0), d_ap(1, 2))
        nc.vector.tensor_mul(sp_ap(5, 1), e_ap(1, 1), d_ap(2, 1))

        # ag = alpha * G  (fp16, [6, ni] c-major)
        ag6 = ag[:].rearrange("p (s i) -> p s i", s=6)
        a_b6 = a16[:, i0 : i0 + ni].unsqueeze(1).broadcast_to((P, 6, ni))
        _eng(nc, ag_e).tensor_mul(ag6, d9[:, 3:9], a_b6)

        # s6 -= ag
        ag_h = ag[:].tensor
        ag_o = ag[:].offset
        ag_w = ag[:].ap[0][0]
        ag_p = bass.AP(ag_h, ag_o,
                       [[ag_w, P], [GROUP, ngr], [1, GROUP], [ni, 6]])
        _eng(nc, sub_e).tensor_sub(sp_ap(0, 6), sp_ap(0, 6), ag_p)

        if half is not None:
            # half-block path (chunk0 split): 2 groups -> partial transpose,
            # 2 matmul groups into a stage tile shared across both halves
            goff = 0 if half == "A" else 2
            st_sb = stp.tile([P, P], F16, tag="stsb")
            st_ps = pools["psq"].tile([P, P], F16, tag="stps")
            nc.tensor.transpose(st_ps[0:64, :], s_pad[:, 0:64], id_sb[:])
            if cfg["ST_ENG"] == "a":
                nc.scalar.copy(st_sb[0:64, :], st_ps[0:64, :])
            else:
                nc.vector.tensor_copy(st_sb[0:64, :], st_ps[0:64, :])
            if half == "A":
                stage = stagep.tile([P, IPB * KG], F16, tag="stage")
                half_stage[0] = stage
            stage = half_stage[0]
            for g2 in range(2):
                o_ps = psm.tile([P, 1024], F32, tag="mmout")
                for h in range(2):
                    nc.tensor.matmul(
                        o_ps[:, 512 * h : 512 * h + NMH],
                        st_sb[32 * g2 : 32 * g2 + 6 * GROUP, :],
                        q_sb[32 * g2 : 32 * g2 + 6 * GROUP,
                             NMH * h : NMH * (h + 1)],
                        start=True,
                        stop=True,
                        tile_position=(32 * g2, 0),
                    )
                eng = "v" if g2 % 2 == 0 else "a"
                mm_idx += 1
                o_h = o_ps[:].tensor
                o_o = o_ps[:].offset
                o_w = o_ps[:].ap[0][0]
                src_ = bass.AP(o_h, o_o, [[o_w, P], [512, 2], [1, NMH]])
                g = goff + g2
                dst = stage[:, NMM * g : NMM * (g + 1)].rearrange(
                    "p (h k) -> p h k", h=2)
                if eng == "a":
                    nc.scalar.copy(dst, src_)
                else:
                    nc.vector.tensor_copy(dst, src_)
            if half == "B":
                out_eng.dma_start(out_flat[:, 0 : IPB * KG], stage[:])
            continue

        # blocks: S^T transpose + 4 row-tiled matmuls + staged copies + DMA
        for lb in range(nb):
            b = blk0 + lb
            st_sb = stp.tile([P, P], F16, tag="stsb")
            s_slice = s_pad[:, P * lb : P * (lb + 1)]
            if cfg["TR_DMA"]:
                nc.scalar.dma_start_transpose(st_sb[:], s_slice)
            else:
                st_ps = pools["psq"].tile([P, P], F16, tag="stps")
                nc.tensor.transpose(st_ps[:], s_slice, id_sb[:])
                if cfg["ST_ENG"] == "a":
                    nc.scalar.copy(st_sb[:], st_ps[:])
                else:
                    nc.vector.tensor_copy(st_sb[:], st_ps[:])

            stage = stagep.tile([P, IPB * KG], F16, tag="stage")
            for g in range(GPB):
                o_ps = psm.tile([P, 1024], F32, tag="mmout")
                for h in range(2):
                    nc.tensor.matmul(
                        o_ps[:, 512 * h : 512 * h + NMH],
                        st_sb[32 * g : 32 * g + 6 * GROUP, :],
                        q_sb[32 * g : 32 * g + 6 * GROUP,
                             NMH * h : NMH * (h + 1)],
                        start=True,
                        stop=True,
                        tile_position=(32 * g, 0),
                    )
                if b == N_BLOCKS - 1 or b < cfg["RAMP_BLOCKS"]:
                    eng = "v" if g % 2 == 0 else "a"  # ramp/tail: both engines
                else:
                    eng = ("v" if (mm_idx * cfg["STAGE_V"]) % 32
                           < cfg["STAGE_V"] else "a")
                mm_idx += 1
                o_h = o_ps[:].tensor
                o_o = o_ps[:].offset
                o_w = o_ps[:].ap[0][0]
                src = bass.AP(o_h, o_o, [[o_w, P], [512, 2], [1, NMH]])
                dst = stage[:, NMM * g : NMM * (g + 1)].rearrange(
                    "p (h k) -> p h k", h=2)
                if eng == "a":
                    nc.scalar.copy(dst, src)
                else:
                    nc.vector.tensor_copy(dst, src)
                if cfg["TAIL_SPLIT"] and b == N_BLOCKS - 1 and g == 1:
                    out_eng.dma_start(
                        out_flat[:, IPB * KG * b : IPB * KG * b + 2 * NMM],
                        stage[:, 0 : 2 * NMM],
                    )
            if cfg["TAIL_SPLIT"] and b == N_BLOCKS - 1:
                out_eng.dma_start(
                    out_flat[:, IPB * KG * b + 2 * NMM : IPB * KG * (b + 1)],
                    stage[:, 2 * NMM :],
                )
            else:
                out_eng.dma_start(
                    out_flat[:, IPB * KG * b : IPB * KG * (b + 1)], stage[:]
                )


_NC_CACHE = {}


def _get_nc(reps=1):
    if reps not in _NC_CACHE:
        _NC_CACHE[reps] = build_nc(reps)
    return _NC_CACHE[reps]


def make_in_maps(eyes, v, R, t, alpha):
    q128 = make_q128()
    ident = np.eye(P, dtype=np.float16)
    eyes = np.ascontiguousarray(eyes, np.float32).reshape(N_CORES, BC, 3)
    v = np.ascontiguousarray(v, np.float32).reshape(N_CORES, BC, 3)
    R = np.ascontiguousarray(R, np.float32).reshape(N_CORES, BC, 3, 3)
    t = np.ascontiguousarray(t, np.float32).reshape(N_CORES, BC, 3)
    alpha = np.ascontiguousarray(alpha, np.float32).reshape(N_CORES, BC)
    return [
        {
            "eyes": eyes[c], "v": v[c], "R": R[c], "t": t[c], "alpha": alpha[c],
            "q128": q128, "ident": ident,
        }
        for c in range(N_CORES)
    ]


def kernel(eyes, v, R, t, alpha):
    nc = _get_nc(1)
    in_maps = make_in_maps(eyes, v, R, t, alpha)
    res = run_bass_kernel_spmd(nc, in_maps, list(range(N_CORES)))
    out = np.concatenate([res.results[c]["out"] for c in range(N_CORES)], axis=0)
    return out.astype(np.float32)
